# revision 48
# baseline (speedup 1.0000x reference)
"""Causal attention layer (N=8, L=2048, H=1024, E=64) on 8 TRN2 NeuronCores.

Sharding: data-parallel over batch N - one batch element per core, Q/K/V
projection weights replicated. No collectives.

Per-core pipeline (93540 -> 56850 ns cost-model time vs the session-start
baseline):
  1. q/k cast-loaded f32->fp8e4m3 (SWDGE) in 512-row chunks; v f32->bf16.
     Load order k0,q0,k1,q1,... then v0..v3 so scores unlock earliest; the
     load stream runs gap-free on the DMA device.
  2. q/k chunks transposed as bf16-VIEWED fp8 PAIRS: 16 PE transposes per
     chunk (half of plain fp8) grouped 8-per-PSUM-bank, one evacuation copy
     per bank; layout [128(h-pair), 4(b), 512(l)] where partition p of
     block b holds h = 256b+2p+{0,1} interleaved.
  3. q/k projections as fp8 DoubleRow matmuls (contract 256 h per matmul,
     0.5 cyc/row): stationary wdr[b] = [128,2(t),64(e)] (de-interleaved W,
     built once via pair-transposes of W); bias added during PSUM evac;
     qpT/kpT stored fp8e4m3 [64, 2048].
  4. Scores ALSO DoubleRow at 0.5 cyc/row via stride-0 broadcast (both
     k-tiles alias the same data => logits doubled; the 1/2 is folded into
     the exp scale). exp on ScalarE; causal mask = upper-tri multiply on
     diagonal blocks; P^T tiles persist in SBUF (bf16). Score pairs are
     emitted lazily (pump) between other PE work so the in-order PE queue
     never head-blocks on the sc<->exp PSUM-recycling ladder.
  5. v0/v1: plain bf16 PE transposes (32/chunk, 8-per-bank groups) -> chv;
     v2/v3: one DmaTransposeAnt each, batched AFTER all loads (single
     DMACopy<->DmaTranspose mode transition; the DMA device idles then).
     Projection -> vpT bf16, PE-transposed (batched) to vaug [128,16,65]
     with a ones column (ctx matmul accumulates softmax row-sums free).
  6. ctx^T[65, i] += vaug[j].T @ P^T[j, i], stripes fully sequential on ONE
     PSUM accumulator (frees a bank so the score pipeline gets 3 PSUM bufs);
     batched epilogue PE-transposes back, divides by row-sum, DMAs out.
"""

import math

import numpy as np

N, L, H, E = 8, 2048, 1024, 64
NCORES = 8
CHUNK = 512
NCHUNK = L // CHUNK  # 4
TPC = CHUNK // 128  # 4 l-tiles per chunk
NBLK = L // 128  # 16 j-blocks
HB = H // 128  # 8

_CACHE = {}

import os
NXBAR_F = int(os.environ.get('KN_NXBAR', '2'))
CONSTS_FIRST = int(os.environ.get('KN_CONSTS_FIRST', '0'))
WCAST_ACT = int(os.environ.get('KN_WCAST_ACT', '0'))
PUMP_N = int(os.environ.get('KN_PUMP', '1'))
SEQCTX = int(os.environ.get('KN_SEQCTX', '1'))
WPREPV_DEFER = int(os.environ.get('KN_WPREPV', '0'))



def _build_nc(reps=1):
    from contextlib import ExitStack

    import concourse.mybir as mybir
    import concourse.tile as tile
    from concourse import bacc
    from concourse.masks import make_identity, make_upper_triangular
    from concourse.tile_rust import add_dep_helper

    f32 = mybir.dt.float32
    bf16 = mybir.dt.bfloat16
    fp8 = mybir.dt.float8e4
    AF = mybir.ActivationFunctionType
    DR = mybir.MatmulPerfMode.DoubleRow
    # DR stride-0 scores double the logit; fold the 1/2 into the exp scale
    scale = 1.0 / (2.0 * math.sqrt(float(L)))

    nc = bacc.Bacc("TRN2", target_bir_lowering=False, debug=False)

    q_ap = nc.dram_tensor("q", [L, H], f32, kind="ExternalInput").ap()
    k_ap = nc.dram_tensor("k", [L, H], f32, kind="ExternalInput").ap()
    v_ap = nc.dram_tensor("v", [L, H], f32, kind="ExternalInput").ap()
    wq_ap = nc.dram_tensor("wq", [E, H], f32, kind="ExternalInput").ap()
    wk_ap = nc.dram_tensor("wk", [E, H], f32, kind="ExternalInput").ap()
    wv_ap = nc.dram_tensor("wv", [E, H], f32, kind="ExternalInput").ap()
    bq_ap = nc.dram_tensor("bq", [E], f32, kind="ExternalInput").ap()
    bk_ap = nc.dram_tensor("bk", [E], f32, kind="ExternalInput").ap()
    bv_ap = nc.dram_tensor("bv", [E], f32, kind="ExternalInput").ap()
    out_ap = nc.dram_tensor("out", [L, E], f32, kind="ExternalOutput").ap()

    x_ap_of = {"q": q_ap, "k": k_ap, "v": v_ap}

    with tile.TileContext(nc) as tc, ExitStack() as ctx:
        const = ctx.enter_context(tc.tile_pool(name="const", bufs=1))
        pTsb = ctx.enter_context(tc.tile_pool(name="pTsb", bufs=1))
        nat8p = ctx.enter_context(tc.tile_pool(name="nat8", bufs=6))
        natvp = ctx.enter_context(tc.tile_pool(name="natv", bufs=4))
        trqp = ctx.enter_context(tc.tile_pool(name="trq", bufs=3))
        chvp = ctx.enter_context(tc.tile_pool(name="chv", bufs=3))
        ptp = ctx.enter_context(tc.tile_pool(name="pt", bufs=1))
        epip = ctx.enter_context(tc.tile_pool(name="epi", bufs=4))
        tpps = ctx.enter_context(tc.tile_pool(name="tpps", bufs=2, space="PSUM"))
        projps = ctx.enter_context(tc.tile_pool(name="projps", bufs=1, space="PSUM"))
        scps = ctx.enter_context(tc.tile_pool(
            name="scps", bufs=(3 if SEQCTX else 2), space="PSUM"))
        ctxps = ctx.enter_context(tc.tile_pool(
            name="ctxps", bufs=(1 if SEQCTX else 2), space="PSUM"))
        smallps = ctx.enter_context(tc.tile_pool(name="smallps", bufs=1, space="PSUM"))

        # ---- persistent tiles ----
        identf = const.tile([128, 128], f32, name="identf")
        identb = const.tile([128, 128], bf16, name="identb")
        tri_f32 = const.tile([128, 128], f32, name="tri_f32")
        tri = const.tile([128, 128], bf16, name="tri")
        b_sb = {}
        wf = {}
        for nm in ("q", "k", "v"):
            b_sb[nm] = const.tile([E, 1], f32, name=f"b_{nm}")
            wf[nm] = const.tile([E, H], f32, name=f"wf_{nm}")
        w8 = {nm: const.tile([E, H], fp8, name=f"w8_{nm}") for nm in ("q", "k")}
        wvb = const.tile([E, H], bf16, name="wvb")
        wdr = {nm: const.tile([128, 4, 2, E], fp8, name=f"wdr_{nm}")
               for nm in ("q", "k")}
        wTv = const.tile([128, HB, E], bf16, name="wTv")

        qpT8 = pTsb.tile([E, L], fp8, name="qpT8")
        kpT8 = pTsb.tile([E, L], fp8, name="kpT8")
        vpT = pTsb.tile([E, L], bf16, name="vpT")
        vaug = pTsb.tile([128, NBLK, E + 1], bf16, name="vaug")
        pT8_of = {"q": qpT8, "k": kpT8}

        def emit_w_loads():
            for nm, w_ap, bias_ap in (("q", wq_ap, bq_ap), ("k", wk_ap, bk_ap),
                                      ("v", wv_ap, bv_ap)):
                nc.scalar.dma_start(out=wf[nm][:], in_=w_ap)
                nc.scalar.dma_start(out=b_sb[nm][:], in_=bias_ap)

        def emit_consts():
            make_identity(nc, identf[:])
            nc.scalar.activation(identb[:], identf[:], AF.Identity)
            # tri[r, c] = 1 where c >= r (key row j <= query col i)
            make_upper_triangular(nc, tri_f32[:], val=1.0, diag=True)
            nc.scalar.activation(tri[:], tri_f32[:], AF.Identity)
            nc.vector.memset(vaug[:, :, E:E + 1], 1.0)

        def emit_w_prep_qk():
            # k's chain on DVE (feeds chunk k0 first), q's on Act; each
            # engine's queue stays short ahead of the first projections
            nc.vector.tensor_copy(w8["k"][:], wf["k"][:])
            nc.scalar.activation(w8["q"][:], wf["q"][:], AF.Identity)
            for nm, copy in (("k", nc.vector.tensor_copy),
                             ("q", lambda d, s: nc.scalar.activation(
                                 d, s, AF.Identity))):
                wu = w8[nm][:].bitcast(bf16)  # [64, 512]
                wps = tpps.tile([128, 2, TPC, 128], bf16, tag="tp",
                                name=f"wps_{nm}")
                for b in range(4):
                    nc.tensor.transpose(
                        wps[:, 0, b, 0:E], wu[:, b * 128:(b + 1) * 128],
                        identb[:E, :E])
                copy(
                    wdr[nm][:],
                    wps[:, 0, :, 0:E].bitcast(fp8).rearrange(
                        "p b (e t) -> p b t e", t=2))

        def emit_w_prep_v():
            nc.vector.tensor_copy(wvb[:], wf["v"][:])
            for g in range(2):
                wps = tpps.tile([128, 2, TPC, 128], bf16, tag="tp",
                                name=f"wvps{g}")
                for i in range(4):
                    hb = g * 4 + i
                    nc.tensor.transpose(
                        wps[:, i // 2, i % 2, 0:E],
                        wvb[:, hb * 128:(hb + 1) * 128], identb[:E, :E])
                nc.vector.tensor_copy(
                    wTv[:, g * 4:(g + 1) * 4].rearrange(
                        "p (a b) e -> p a b e", a=2),
                    wps[:, :, 0:2, 0:E])

        def emit_load(nm, c):
            l0 = c * CHUNK
            dtt = bf16 if nm == "v" else fp8
            pool = natvp if nm == "v" else nat8p
            nat = pool.tile([128, TPC, H], dtt, tag="nat", name=f"nat_{nm}{c}")
            src = x_ap_of[nm][l0:l0 + CHUNK, :].rearrange("(t p) h -> p t h", p=128)
            ld = nc.gpsimd.dma_start(out=nat[:].rearrange("p t h -> p (t h)"),
                                     in_=src)
            return nat, ld

        # ---- q/k chunk: pair transposes (2 groups of 8) + DR projection ----
        def emit_qk_tp_proj(nm, c, nat):
            l0 = c * CHUNK
            natu = nat[:].bitcast(bf16)  # [128, TPC, 512] u16 pairs
            trq = trqp.tile([128, 4, CHUNK], bf16, tag="trq", name=f"trq_{nm}{c}")
            for g in range(2):  # lt in {2g, 2g+1}
                tps = tpps.tile([128, 2, 4, 128], bf16, tag="tp",
                                name=f"tp_{nm}{c}{g}")
                for i in range(2):
                    lt = 2 * g + i
                    for b in range(4):
                        nc.tensor.transpose(
                            tps[:, i, b], natu[:, lt, b * 128:(b + 1) * 128],
                            identb[:])
                # [128, (lt2, b, l)] -> trq[:, b, (2g+i)*128 + l]
                nc.vector.tensor_copy(
                    trq[:, :, g * 256:(g + 1) * 256].rearrange(
                        "p b (i l) -> p i b l", i=2),
                    tps[:])
                pump(PUMP_N)
            pj = projps.tile([E, CHUNK], f32, tag="pj", name=f"pj_{nm}{c}")
            for b in range(4):
                nc.tensor.matmul(
                    pj[:],
                    lhsT=wdr[nm][:, b],
                    rhs=trq[:, b].bitcast(fp8).rearrange("p (l t) -> p t l", t=2),
                    start=(b == 0),
                    stop=(b == 3),
                    perf_mode=DR,
                )
            nc.scalar.activation(pT8_of[nm][:, l0:l0 + CHUNK], pj[:], AF.Identity,
                                 bias=b_sb[nm][:])
            pump(1)

        # ---- scores (DoubleRow stride-0), emitted lazily via pump() so the
        # in-order PE queue never head-blocks on the exp ladder ----
        pt_info = {}
        pending = []  # (s, pi, take) score pairs not yet emitted

        def dr2(ap):
            return ap.rearrange("p (o l) -> p o l", o=1).broadcast_to(
                (ap.shape[0], 2, ap.shape[-1]))

        def queue_scores(s):
            i0, i1 = s * CHUNK, (s + 1) * CHUNK
            js = list(range(4 * s + 4))
            pi = 0
            pt_info[s] = []
            while js:
                take = js[:1]
                w0 = i1 - max(i0, js[0] * 128)
                if len(js) > 1 and w0 + (i1 - max(i0, js[1] * 128)) <= 512:
                    take = js[:2]
                pending.append((s, pi, take))
                pi += 1
                js = js[len(take):]

        def pump(n):
            for _ in range(min(n, len(pending))):
                s, pi, take = pending.pop(0)
                i0, i1 = s * CHUNK, (s + 1) * CHUNK
                sc = scps.tile([128, 512], f32, tag="sc", name=f"sc_{s}_{pi}")
                pt = ptp.tile([128, 512], bf16, tag=f"pt_{s}_{pi}",
                              name=f"pt_{s}_{pi}")
                infos = []
                off = 0
                for j in take:
                    g0 = max(i0, j * 128)
                    w = i1 - g0
                    nc.tensor.matmul(
                        sc[:, off:off + w],
                        lhsT=dr2(kpT8[:, j * 128:(j + 1) * 128]),
                        rhs=dr2(qpT8[:, g0:g0 + w]),
                        start=True,
                        stop=True,
                        perf_mode=DR,
                    )
                    infos.append((j, g0, w, off))
                    off += w
                nc.scalar.activation(pt[:, 0:off], sc[:, 0:off], AF.Exp,
                                     scale=scale)
                for j, g0, w, off_ in infos:
                    if g0 == j * 128:  # diagonal block: causal mask
                        nc.vector.tensor_mul(
                            pt[:, off_:off_ + 128], pt[:, off_:off_ + 128],
                            tri[:])
                pt_info[s].append((pt, infos))

        def drain_stripe(s):
            while pending and pending[0][0] <= s:
                pump(1)

        # ---- v chunk ----
        # c < NXBAR: plain PE transposes (Act/DVE have slack early).
        # c >= NXBAR: one DmaTranspose per chunk, batched AFTER all loads
        # (single DMACopy<->DmaTranspose mode transition; DMA idles then).
        NXBAR = NXBAR_F
        def emit_v_chunk(c, nat, last_ld, prev_xb):
            l0 = c * CHUNK
            xb = None
            if c >= NXBAR:
                cht = chvp.tile([128, HB * TPC, 128], bf16, tag="chx",
                                name=f"chx{c}")
                xb = nc.sync.dma_start(
                    out=cht[:],
                    in_=nat[:].rearrange("p t h -> p (t h)"),
                    transpose=True,
                )
                add_dep_helper(xb.ins, last_ld.ins, sync=True,
                               reason="batch v xbars after all loads")
                # block b of cht = (lt, hb): rhs for hb = [128, lt, 128]
                chb = cht[:].rearrange("p (lt hb) l -> p lt hb l", lt=TPC, hb=HB)
                rhs_of = lambda hb: chb[:, :, hb, :]
            else:
                chv = chvp.tile([128, HB, CHUNK], bf16, tag="chv",
                                name=f"chv{c}")
                for g in range(4):  # hb in {2g, 2g+1}
                    vt = tpps.tile([128, 2, TPC, 128], bf16, tag="tp",
                                   name=f"vt{c}{g}")
                    for i in range(2):
                        hb = 2 * g + i
                        for lt in range(TPC):
                            nc.tensor.transpose(
                                vt[:, i, lt],
                                nat[:, lt, hb * 128:(hb + 1) * 128],
                                identb[:])
                    nc.vector.tensor_copy(chv[:, 2 * g:2 * g + 2], vt[:])
                    pump(1)
                rhs_of = lambda hb: chv[:, hb]
            pj = projps.tile([E, CHUNK], f32, tag="pj", name=f"pjv{c}")
            for hb in range(HB):
                nc.tensor.matmul(
                    pj[:],
                    lhsT=wTv[:, hb],
                    rhs=rhs_of(hb),
                    start=(hb == 0),
                    stop=(hb == HB - 1),
                )
            nc.vector.tensor_scalar_add(vpT[:, l0:l0 + CHUNK], pj[:], b_sb["v"][:])
            # batched vaug: 4 transposes into one PSUM tile, one copy out
            sm = smallps.tile([128, 4, E + 4], f32, tag="sm", name=f"vaugtp{c}")
            vps = sm[:].bitcast(bf16)  # [128, 4, 2*(E+4)]
            for t in range(TPC):
                jb = c * TPC + t
                nc.tensor.transpose(
                    vps[:, t, :E], vpT[:, jb * 128:(jb + 1) * 128],
                    identb[:E, :E])
            nc.vector.tensor_copy(
                vaug[:, c * TPC:(c + 1) * TPC, 0:E], vps[:, :, 0:E])
            return xb

        # ---- ctx accumulation ----
        ctx_of = {}

        def emit_ctx(s, c):
            drain_stripe(s)
            assert not any(p[0] <= s for p in pending)
            i0 = s * CHUNK
            jmax = 4 * s + 3
            if s not in ctx_of:
                ctx_of[s] = ctxps.tile([E + 1, CHUNK], f32, tag="ctx",
                                       name=f"ctx{s}")
            ctx_ps = ctx_of[s]
            jlo, jhi = 4 * c, min(4 * c + 3, jmax)
            for pt, infos in pt_info[s]:
                for j, g0, w, off in infos:
                    if not (jlo <= j <= jhi):
                        continue
                    nc.tensor.matmul(
                        ctx_ps[:, g0 - i0:g0 - i0 + w],
                        lhsT=vaug[:, j],
                        rhs=pt[:, off:off + w],
                        start=(j == 0),
                        stop=(j == jmax),
                    )

        def emit_epi(s):
            i0, i1 = s * CHUNK, (s + 1) * CHUNK
            late = s >= 2
            ctx_ps = ctx_of[s]
            ctxsb = epip.tile([E + 1, CHUNK], f32, tag="ctxsb", name=f"ctxsb{s}")
            nc.vector.tensor_copy(ctxsb[:], ctx_ps[:])
            outsb = epip.tile([128, TPC, E], f32, tag="outsb", name=f"outsb{s}")
            # batched epilogue: 4 transposes into one PSUM tile, one recip,
            # then 4 back-to-back muls
            cps = smallps.tile([128, 4, E + 4], f32, tag="sm", name=f"etp{s}")
            for t in range(TPC):
                nc.tensor.transpose(
                    cps[:, t, 0:E + 1],
                    ctxsb[:, t * 128:(t + 1) * 128],
                    identf[:E + 1, :E + 1],
                )
            rec = epip.tile([128, TPC], f32, tag="rec", name=f"rec{s}")
            nc.vector.reciprocal(rec[:], cps[:, :, E:E + 1])
            for t in range(TPC):
                nc.vector.tensor_scalar_mul(outsb[:, t, :], cps[:, t, 0:E],
                                            rec[:, t:t + 1])
            dst = out_ap[i0:i1, :].rearrange("(t p) e -> p t e", p=128)
            nc.scalar.dma_start(out=dst, in_=outsb[:])

        # ================= emission schedule =================
        for _ in range(reps):
            pt_info.clear()
            ctx_of.clear()
            del pending[:]
            if CONSTS_FIRST:
                emit_consts()  # Pool consts first: DMA init hides them
            emit_w_loads()
            nats = {}
            nats[("k", 0)] = emit_load("k", 0)[0]
            nats[("q", 0)] = emit_load("q", 0)[0]
            if not CONSTS_FIRST:
                emit_consts()
            emit_w_prep_qk()
            if not WPREPV_DEFER:
                emit_w_prep_v()
            for c in range(NCHUNK):
                if c > 0:
                    nats[("k", c)] = emit_load("k", c)[0]
                    nats[("q", c)] = emit_load("q", c)[0]
                emit_qk_tp_proj("k", c, nats[("k", c)])
                emit_qk_tp_proj("q", c, nats[("q", c)])
                queue_scores(c)
                if c == 0 and WPREPV_DEFER:
                    emit_w_prep_v()
            vloads = [emit_load("v", c) for c in range(NCHUNK)]
            last_ld = vloads[-1][1]
            # v blocks with interleaved ctx; stripe pairing keeps ctxps at 2
            # bufs without in-order-queue deadlock (epi(s) is emitted before
            # any stripe that reuses its PSUM accumulator).
            prev_xb = None
            for c in range(NCHUNK):
                prev_xb = emit_v_chunk(c, vloads[c][0], last_ld, prev_xb)
                if SEQCTX:
                    # fully sequential stripes: one ctx accumulator
                    for cc in range(c + 1):
                        emit_ctx(c, cc)
                    emit_epi(c)
                elif c == 0:
                    emit_ctx(0, 0)
                    emit_epi(0)
                    emit_ctx(1, 0)
                    emit_ctx(2, 0)
                elif c == 1:
                    emit_ctx(1, 1)
                    emit_epi(1)
                    emit_ctx(2, 1)
                    emit_ctx(3, 0)
                    emit_ctx(3, 1)
                elif c == 2:
                    emit_ctx(2, 2)
                    emit_epi(2)
                    emit_ctx(3, 2)
                else:
                    emit_ctx(3, 3)
                    emit_epi(3)

    nc.compile()
    return nc


def _get_nc(reps=1):
    key = ("nc", reps)
    if key not in _CACHE:
        _CACHE[key] = _build_nc(reps)
    return _CACHE[key]


def kernel(q, k, v, key_padding_mask=None, Wq=None, bq=None, Wk=None, bk=None,
           Wv=None, bv=None):
    from concourse.bass_utils import run_bass_kernel_spmd

    nc = _get_nc()
    f = np.float32
    shared = {
        "wq": np.ascontiguousarray(Wq, dtype=f),
        "wk": np.ascontiguousarray(Wk, dtype=f),
        "wv": np.ascontiguousarray(Wv, dtype=f),
        "bq": np.ascontiguousarray(bq, dtype=f),
        "bk": np.ascontiguousarray(bk, dtype=f),
        "bv": np.ascontiguousarray(bv, dtype=f),
    }
    in_maps = []
    for n in range(NCORES):
        m = dict(shared)
        m["q"] = np.ascontiguousarray(q[n], dtype=f)
        m["k"] = np.ascontiguousarray(k[n], dtype=f)
        m["v"] = np.ascontiguousarray(v[n], dtype=f)
        in_maps.append(m)
    res = run_bass_kernel_spmd(nc, in_maps, core_ids=list(range(NCORES)))
    out = np.stack([res.results[i]["out"] for i in range(NCORES)], axis=0)
    return out.astype(np.float32)


# revision 71
# speedup vs baseline: 1.0276x; 1.0276x over previous
"""Causal attention layer (N=8, L=2048, H=1024, E=64) on 8 TRN2 NeuronCores.

Sharding: data-parallel over batch N - one batch element per core, Q/K/V
projection weights replicated. No collectives.

Per-core pipeline (93540 -> 55324 ns cost-model time vs the session-start
baseline):
  1. q/k cast-loaded f32->fp8e4m3 (SWDGE) in 512-row chunks; v f32->bf16.
     Load order k0,q0,k1,q1,... then v0..v3 so scores unlock earliest; the
     load stream runs gap-free on the DMA device.
  2. q/k chunks transposed as bf16-VIEWED fp8 PAIRS: 16 PE transposes per
     chunk (half of plain fp8) grouped 8-per-PSUM-bank, one evacuation copy
     per bank; layout [128(h-pair), 4(b), 512(l)] where partition p of
     block b holds h = 256b+2p+{0,1} interleaved.
  3. q/k projections as fp8 DoubleRow matmuls (contract 256 h per matmul,
     0.5 cyc/row): stationary wdr[b] = [128,2(t),64(e)] (de-interleaved W,
     built once via pair-transposes of W); bias added during PSUM evac;
     qpT/kpT stored fp8e4m3 [64, 2048].
  4. Scores ALSO DoubleRow at 0.5 cyc/row via stride-0 broadcast (both
     k-tiles alias the same data => logits doubled; the 1/2 is folded into
     the exp scale). exp on ScalarE; causal mask = upper-tri multiply on
     diagonal blocks; P^T tiles persist in SBUF (bf16). Score pairs are
     emitted lazily (pump) between other PE work so the in-order PE queue
     never head-blocks on the sc<->exp PSUM-recycling ladder.
  5. v0/v1: plain bf16 PE transposes (32/chunk, 8-per-bank groups) -> chv;
     v2/v3: one DmaTransposeAnt each, batched AFTER all loads (single
     DMACopy<->DmaTranspose mode transition; the DMA device idles then).
     Projection -> vpT bf16, PE-transposed (batched) to vaug [128,16,65]
     with a ones column (ctx matmul accumulates softmax row-sums free).
  6. ctx^T[65, i] += vaug[j].T @ P^T[j, i], stripes fully sequential on ONE
     PSUM accumulator (frees a bank so the score pipeline gets 3 PSUM bufs);
     batched epilogue PE-transposes back, divides by row-sum, DMAs out.
"""

import math

import numpy as np

N, L, H, E = 8, 2048, 1024, 64
NCORES = 8
CHUNK = 512
NCHUNK = L // CHUNK  # 4
TPC = CHUNK // 128  # 4 l-tiles per chunk
NBLK = L // 128  # 16 j-blocks
HB = H // 128  # 8

_CACHE = {}

import os
NXBAR_F = int(os.environ.get('KN_NXBAR', '2'))
CONSTS_FIRST = int(os.environ.get('KN_CONSTS_FIRST', '0'))
WCAST_ACT = int(os.environ.get('KN_WCAST_ACT', '0'))
PUMP_N = int(os.environ.get('KN_PUMP', '1'))
SEQCTX = int(os.environ.get('KN_SEQCTX', '1'))
WPREPV_DEFER = int(os.environ.get('KN_WPREPV', '0'))
VORDER = os.environ.get('KN_VORDER', '0123')
PJEVAC_DVE = int(os.environ.get('KN_PJEVAC', '0'))
DRAINC = int(os.environ.get('KN_DRAINC', '0'))
BUFS = [int(x) for x in os.environ.get('KN_BUFS', '6,4,3,3').split(',')]



def _build_nc(reps=1):
    from contextlib import ExitStack

    import concourse.mybir as mybir
    import concourse.tile as tile
    from concourse import bacc
    from concourse.masks import make_identity, make_upper_triangular
    from concourse.tile_rust import add_dep_helper

    f32 = mybir.dt.float32
    bf16 = mybir.dt.bfloat16
    fp8 = mybir.dt.float8e4
    AF = mybir.ActivationFunctionType
    DR = mybir.MatmulPerfMode.DoubleRow
    # DR stride-0 scores double the logit; fold the 1/2 into the exp scale
    scale = 1.0 / (2.0 * math.sqrt(float(L)))

    nc = bacc.Bacc("TRN2", target_bir_lowering=False, debug=False)

    q_ap = nc.dram_tensor("q", [L, H], f32, kind="ExternalInput").ap()
    k_ap = nc.dram_tensor("k", [L, H], f32, kind="ExternalInput").ap()
    v_ap = nc.dram_tensor("v", [L, H], f32, kind="ExternalInput").ap()
    wq_ap = nc.dram_tensor("wq", [E, H], f32, kind="ExternalInput").ap()
    wk_ap = nc.dram_tensor("wk", [E, H], f32, kind="ExternalInput").ap()
    wv_ap = nc.dram_tensor("wv", [E, H], f32, kind="ExternalInput").ap()
    bq_ap = nc.dram_tensor("bq", [E], f32, kind="ExternalInput").ap()
    bk_ap = nc.dram_tensor("bk", [E], f32, kind="ExternalInput").ap()
    bv_ap = nc.dram_tensor("bv", [E], f32, kind="ExternalInput").ap()
    out_ap = nc.dram_tensor("out", [L, E], f32, kind="ExternalOutput").ap()

    x_ap_of = {"q": q_ap, "k": k_ap, "v": v_ap}

    with tile.TileContext(nc) as tc, ExitStack() as ctx:
        const = ctx.enter_context(tc.tile_pool(name="const", bufs=1))
        pTsb = ctx.enter_context(tc.tile_pool(name="pTsb", bufs=1))
        nat8p = ctx.enter_context(tc.tile_pool(name="nat8", bufs=BUFS[0]))
        natvp = ctx.enter_context(tc.tile_pool(name="natv", bufs=BUFS[1]))
        trqp = ctx.enter_context(tc.tile_pool(name="trq", bufs=BUFS[2]))
        chvp = ctx.enter_context(tc.tile_pool(name="chv", bufs=BUFS[3]))
        ptp = ctx.enter_context(tc.tile_pool(name="pt", bufs=1))
        epip = ctx.enter_context(tc.tile_pool(name="epi", bufs=4))
        tpps = ctx.enter_context(tc.tile_pool(name="tpps", bufs=2, space="PSUM"))
        projps = ctx.enter_context(tc.tile_pool(name="projps", bufs=1, space="PSUM")) if SEQCTX != 2 else None
        scps = ctx.enter_context(tc.tile_pool(
            name="scps", bufs=(3 if SEQCTX else 2), space="PSUM"))
        ctxps = ctx.enter_context(tc.tile_pool(
            name="ctxps", bufs=(1 if SEQCTX in (1, 4) else 2), space="PSUM"))
        smallps = ctx.enter_context(tc.tile_pool(name="smallps", bufs=1, space="PSUM"))

        # ---- persistent tiles ----
        identf = const.tile([128, 128], f32, name="identf")
        identb = const.tile([128, 128], bf16, name="identb")
        tri_f32 = const.tile([128, 128], f32, name="tri_f32")
        tri = const.tile([128, 128], bf16, name="tri")
        b_sb = {}
        wf = {}
        for nm in ("q", "k", "v"):
            b_sb[nm] = const.tile([E, 1], f32, name=f"b_{nm}")
            wf[nm] = const.tile([E, H], f32, name=f"wf_{nm}")
        w8 = {nm: const.tile([E, H], fp8, name=f"w8_{nm}") for nm in ("q", "k")}
        wvb = const.tile([E, H], bf16, name="wvb")
        wdr = {nm: const.tile([128, 4, 2, E], fp8, name=f"wdr_{nm}")
               for nm in ("q", "k")}
        wTv = const.tile([128, HB, E], bf16, name="wTv")

        qpT8 = pTsb.tile([E, L], fp8, name="qpT8")
        kpT8 = pTsb.tile([E, L], fp8, name="kpT8")
        vpT = pTsb.tile([E, L], bf16, name="vpT")
        vaug = pTsb.tile([128, NBLK, E + 1], bf16, name="vaug")
        pT8_of = {"q": qpT8, "k": kpT8}

        def emit_w_loads():
            for nm, w_ap, bias_ap in (("q", wq_ap, bq_ap), ("k", wk_ap, bk_ap),
                                      ("v", wv_ap, bv_ap)):
                nc.scalar.dma_start(out=wf[nm][:], in_=w_ap)
                nc.scalar.dma_start(out=b_sb[nm][:], in_=bias_ap)

        def emit_consts():
            make_identity(nc, identf[:])
            nc.scalar.activation(identb[:], identf[:], AF.Identity)
            # tri[r, c] = 1 where c >= r (key row j <= query col i)
            make_upper_triangular(nc, tri_f32[:], val=1.0, diag=True)
            nc.scalar.activation(tri[:], tri_f32[:], AF.Identity)
            nc.vector.memset(vaug[:, :, E:E + 1], 1.0)

        def emit_w_prep_qk():
            # k's chain on DVE (feeds chunk k0 first), q's on Act; each
            # engine's queue stays short ahead of the first projections
            nc.vector.tensor_copy(w8["k"][:], wf["k"][:])
            nc.scalar.activation(w8["q"][:], wf["q"][:], AF.Identity)
            for nm, copy in (("k", nc.vector.tensor_copy),
                             ("q", lambda d, s: nc.scalar.activation(
                                 d, s, AF.Identity))):
                wu = w8[nm][:].bitcast(bf16)  # [64, 512]
                wps = tpps.tile([128, 2, TPC, 128], bf16, tag="tp",
                                name=f"wps_{nm}")
                for b in range(4):
                    nc.tensor.transpose(
                        wps[:, 0, b, 0:E], wu[:, b * 128:(b + 1) * 128],
                        identb[:E, :E])
                copy(
                    wdr[nm][:],
                    wps[:, 0, :, 0:E].bitcast(fp8).rearrange(
                        "p b (e t) -> p b t e", t=2))

        def emit_w_prep_v():
            nc.vector.tensor_copy(wvb[:], wf["v"][:])
            for g in range(2):
                wps = tpps.tile([128, 2, TPC, 128], bf16, tag="tp",
                                name=f"wvps{g}")
                for i in range(4):
                    hb = g * 4 + i
                    nc.tensor.transpose(
                        wps[:, i // 2, i % 2, 0:E],
                        wvb[:, hb * 128:(hb + 1) * 128], identb[:E, :E])
                nc.vector.tensor_copy(
                    wTv[:, g * 4:(g + 1) * 4].rearrange(
                        "p (a b) e -> p a b e", a=2),
                    wps[:, :, 0:2, 0:E])

        def emit_load(nm, c):
            l0 = c * CHUNK
            dtt = bf16 if nm == "v" else fp8
            pool = natvp if nm == "v" else nat8p
            nat = pool.tile([128, TPC, H], dtt, tag="nat", name=f"nat_{nm}{c}")
            src = x_ap_of[nm][l0:l0 + CHUNK, :].rearrange("(t p) h -> p t h", p=128)
            ld = nc.gpsimd.dma_start(out=nat[:].rearrange("p t h -> p (t h)"),
                                     in_=src)
            return nat, ld

        # ---- q/k chunk: pair transposes (2 groups of 8) + DR projection ----
        def emit_qk_tp_proj(nm, c, nat):
            l0 = c * CHUNK
            natu = nat[:].bitcast(bf16)  # [128, TPC, 512] u16 pairs
            trq = trqp.tile([128, 4, CHUNK], bf16, tag="trq", name=f"trq_{nm}{c}")
            for g in range(2):  # lt in {2g, 2g+1}
                tps = tpps.tile([128, 2, 4, 128], bf16, tag="tp",
                                name=f"tp_{nm}{c}{g}")
                for i in range(2):
                    lt = 2 * g + i
                    for b in range(4):
                        nc.tensor.transpose(
                            tps[:, i, b], natu[:, lt, b * 128:(b + 1) * 128],
                            identb[:])
                # [128, (lt2, b, l)] -> trq[:, b, (2g+i)*128 + l]
                nc.vector.tensor_copy(
                    trq[:, :, g * 256:(g + 1) * 256].rearrange(
                        "p b (i l) -> p i b l", i=2),
                    tps[:])
                pump(PUMP_N)
            if SEQCTX == 2:
                pj = scps.tile([128, 512], f32, tag="sc",
                               name=f"pj_{nm}{c}")[:E, :]
            else:
                pj = projps.tile([E + 1, CHUNK], f32, tag="pj",
                                 name=f"pj_{nm}{c}")[0:E, :]
            for b in range(4):
                nc.tensor.matmul(
                    pj[:],
                    lhsT=wdr[nm][:, b],
                    rhs=trq[:, b].bitcast(fp8).rearrange("p (l t) -> p t l", t=2),
                    start=(b == 0),
                    stop=(b == 3),
                    perf_mode=DR,
                )
            if PJEVAC_DVE:
                nc.vector.tensor_scalar_add(pT8_of[nm][:, l0:l0 + CHUNK],
                                            pj[:], b_sb[nm][:])
            else:
                nc.scalar.activation(pT8_of[nm][:, l0:l0 + CHUNK], pj[:],
                                     AF.Identity, bias=b_sb[nm][:])
            pump(1)

        # ---- scores (DoubleRow stride-0), emitted lazily via pump() so the
        # in-order PE queue never head-blocks on the exp ladder ----
        pt_info = {}
        pending = []  # (s, pi, take) score pairs not yet emitted

        def dr2(ap):
            return ap.rearrange("p (o l) -> p o l", o=1).broadcast_to(
                (ap.shape[0], 2, ap.shape[-1]))

        def queue_scores(s):
            i0, i1 = s * CHUNK, (s + 1) * CHUNK
            js = list(range(4 * s + 4))
            pi = 0
            pt_info[s] = []
            while js:
                take = js[:1]
                w0 = i1 - max(i0, js[0] * 128)
                if len(js) > 1 and w0 + (i1 - max(i0, js[1] * 128)) <= 512:
                    take = js[:2]
                pending.append((s, pi, take))
                pi += 1
                js = js[len(take):]

        def pump(n):
            for _ in range(min(n, len(pending))):
                s, pi, take = pending.pop(0)
                i0, i1 = s * CHUNK, (s + 1) * CHUNK
                sc = scps.tile([128, 512], f32, tag="sc", name=f"sc_{s}_{pi}")
                pt = ptp.tile([128, 512], bf16, tag=f"pt_{s}_{pi}",
                              name=f"pt_{s}_{pi}")
                infos = []
                off = 0
                for j in take:
                    g0 = max(i0, j * 128)
                    w = i1 - g0
                    nc.tensor.matmul(
                        sc[:, off:off + w],
                        lhsT=dr2(kpT8[:, j * 128:(j + 1) * 128]),
                        rhs=dr2(qpT8[:, g0:g0 + w]),
                        start=True,
                        stop=True,
                        perf_mode=DR,
                    )
                    infos.append((j, g0, w, off))
                    off += w
                nc.scalar.activation(pt[:, 0:off], sc[:, 0:off], AF.Exp,
                                     scale=scale)
                for j, g0, w, off_ in infos:
                    if g0 == j * 128:  # diagonal block: causal mask
                        nc.vector.tensor_mul(
                            pt[:, off_:off_ + 128], pt[:, off_:off_ + 128],
                            tri[:])
                pt_info[s].append((pt, infos))

        def drain_stripe(s):
            while pending and pending[0][0] <= s:
                pump(1)

        # ---- v chunk ----
        # c < NXBAR: plain PE transposes (Act/DVE have slack early).
        # c >= NXBAR: one DmaTranspose per chunk, batched AFTER all loads
        # (single DMACopy<->DmaTranspose mode transition; DMA idles then).
        NXBAR = NXBAR_F
        def emit_v_chunk(c, nat, last_ld, prev_xb):
            l0 = c * CHUNK
            xb = None
            if c >= NXBAR:
                cht = chvp.tile([128, HB * TPC, 128], bf16, tag="chx",
                                name=f"chx{c}")
                xb = nc.sync.dma_start(
                    out=cht[:],
                    in_=nat[:].rearrange("p t h -> p (t h)"),
                    transpose=True,
                )
                add_dep_helper(xb.ins, last_ld.ins, sync=True,
                               reason="batch v xbars after all loads")
                # block b of cht = (lt, hb): rhs for hb = [128, lt, 128]
                chb = cht[:].rearrange("p (lt hb) l -> p lt hb l", lt=TPC, hb=HB)
                rhs_of = lambda hb: chb[:, :, hb, :]
            else:
                chv = chvp.tile([128, HB, CHUNK], bf16, tag="chv",
                                name=f"chv{c}")
                for g in range(4):  # hb in {2g, 2g+1}
                    vt = tpps.tile([128, 2, TPC, 128], bf16, tag="tp",
                                   name=f"vt{c}{g}")
                    for i in range(2):
                        hb = 2 * g + i
                        for lt in range(TPC):
                            nc.tensor.transpose(
                                vt[:, i, lt],
                                nat[:, lt, hb * 128:(hb + 1) * 128],
                                identb[:])
                    nc.vector.tensor_copy(chv[:, 2 * g:2 * g + 2], vt[:])
                    pump(1)
                rhs_of = lambda hb: chv[:, hb]
            if SEQCTX == 2:
                pj = scps.tile([128, 512], f32, tag="sc",
                               name=f"pjv{c}")[:E, :]
            else:
                pj = projps.tile([E + 1, CHUNK], f32, tag="pj",
                                 name=f"pjv{c}")[0:E, :]
            for hb in range(HB):
                nc.tensor.matmul(
                    pj[:],
                    lhsT=wTv[:, hb],
                    rhs=rhs_of(hb),
                    start=(hb == 0),
                    stop=(hb == HB - 1),
                )
            nc.vector.tensor_scalar_add(vpT[:, l0:l0 + CHUNK], pj[:], b_sb["v"][:])
            # batched vaug: 4 transposes into one PSUM tile, one copy out
            sm = smallps.tile([128, 4, E + 4], f32, tag="sm", name=f"vaugtp{c}")
            vps = sm[:].bitcast(bf16)  # [128, 4, 2*(E+4)]
            for t in range(TPC):
                jb = c * TPC + t
                nc.tensor.transpose(
                    vps[:, t, :E], vpT[:, jb * 128:(jb + 1) * 128],
                    identb[:E, :E])
            nc.vector.tensor_copy(
                vaug[:, c * TPC:(c + 1) * TPC, 0:E], vps[:, :, 0:E])
            return xb

        # ---- ctx accumulation ----
        ctx_of = {}

        def emit_ctx(s, c):
            drain_stripe(s)
            assert not any(p[0] <= s for p in pending)
            i0 = s * CHUNK
            jmax = 4 * s + 3
            if s not in ctx_of:
                if SEQCTX == 4 and s == NCHUNK - 1:
                    ctx_of[s] = projps.tile([E + 1, CHUNK], f32, tag="pj",
                                            name=f"ctx{s}")
                else:
                    ctx_of[s] = ctxps.tile([E + 1, CHUNK], f32, tag="ctx",
                                           name=f"ctx{s}")
            ctx_ps = ctx_of[s]
            jlo, jhi = 4 * c, min(4 * c + 3, jmax)
            for pt, infos in pt_info[s]:
                for j, g0, w, off in infos:
                    if not (jlo <= j <= jhi):
                        continue
                    nc.tensor.matmul(
                        ctx_ps[:, g0 - i0:g0 - i0 + w],
                        lhsT=vaug[:, j],
                        rhs=pt[:, off:off + w],
                        start=(j == 0),
                        stop=(j == jmax),
                    )

        def emit_epi(s):
            i0, i1 = s * CHUNK, (s + 1) * CHUNK
            late = s >= 2
            ctx_ps = ctx_of[s]
            ctxsb = epip.tile([E + 1, CHUNK], f32, tag="ctxsb", name=f"ctxsb{s}")
            nc.vector.tensor_copy(ctxsb[:], ctx_ps[:])
            outsb = epip.tile([128, TPC, E], f32, tag="outsb", name=f"outsb{s}")
            # batched epilogue: 4 transposes into one PSUM tile, one recip,
            # then 4 back-to-back muls
            cps = smallps.tile([128, 4, E + 4], f32, tag="sm", name=f"etp{s}")
            for t in range(TPC):
                nc.tensor.transpose(
                    cps[:, t, 0:E + 1],
                    ctxsb[:, t * 128:(t + 1) * 128],
                    identf[:E + 1, :E + 1],
                )
            rec = epip.tile([128, TPC], f32, tag="rec", name=f"rec{s}")
            nc.vector.reciprocal(rec[:], cps[:, :, E:E + 1])
            # one broadcast multiply instead of four: rec stride-0 along e
            nc.vector.tensor_mul(
                outsb[:],
                cps[:, :, 0:E],
                rec[:].rearrange("p (t o) -> p t o", o=1).broadcast_to(
                    (128, TPC, E)))
            dst = out_ap[i0:i1, :].rearrange("(t p) e -> p t e", p=128)
            nc.scalar.dma_start(out=dst, in_=outsb[:])

        # ================= emission schedule =================
        for _ in range(reps):
            pt_info.clear()
            ctx_of.clear()
            del pending[:]
            if CONSTS_FIRST:
                emit_consts()  # Pool consts first: DMA init hides them
            emit_w_loads()
            nats = {}
            nats[("k", 0)] = emit_load("k", 0)[0]
            nats[("q", 0)] = emit_load("q", 0)[0]
            if not CONSTS_FIRST:
                emit_consts()
            emit_w_prep_qk()
            if not WPREPV_DEFER:
                emit_w_prep_v()
            for c in range(NCHUNK):
                if c > 0:
                    nats[("k", c)] = emit_load("k", c)[0]
                    nats[("q", c)] = emit_load("q", c)[0]
                emit_qk_tp_proj("k", c, nats[("k", c)])
                emit_qk_tp_proj("q", c, nats[("q", c)])
                queue_scores(c)
                if DRAINC == -2:
                    drain_stripe(c)  # emit each stripe's scores immediately
                if c == 0 and WPREPV_DEFER:
                    emit_w_prep_v()
            if DRAINC < 0:
                drain_stripe(NCHUNK - 1)  # drain all scores pre-v-phase
            vorder = [int(x) for x in VORDER]
            vload_map = {}
            for c in vorder:
                vload_map[c] = emit_load("v", c)
            vloads = [vload_map[c] for c in range(NCHUNK)]
            last_ld = vload_map[vorder[-1]][1]
            # v blocks with interleaved ctx; stripe pairing keeps ctxps at 2
            # bufs without in-order-queue deadlock (epi(s) is emitted before
            # any stripe that reuses its PSUM accumulator).
            prev_xb = None
            for c in range(NCHUNK):
                prev_xb = emit_v_chunk(c, vloads[c][0], last_ld, prev_xb)
                if SEQCTX in (1, 4):
                    # stripes sequential on one ctx accumulator; in mode 4
                    # the LAST stripe instead accumulates in the projection
                    # bank (free once v3's projection evacuates), in
                    # parallel with stripe 2's chain
                    if SEQCTX == 4 and c == NCHUNK - 1:
                        pass  # stripe 3 handled below on projps
                    else:
                        for cc in range(c + 1):
                            emit_ctx(c, cc)
                        emit_epi(c)
                    if SEQCTX == 4 and c == NCHUNK - 1:
                        for cc in range(NCHUNK):
                            emit_ctx(NCHUNK - 1, cc)
                        emit_epi(NCHUNK - 1)
                    if c == max(DRAINC, 0):
                        # all remaining score pairs must be in the PE queue
                        # BEFORE the v2/v3 xbar-gated work, or their exps
                        # gate the final stripe's context tail
                        drain_stripe(NCHUNK - 1)
                elif c == 0:
                    emit_ctx(0, 0)
                    emit_epi(0)
                    emit_ctx(1, 0)
                    emit_ctx(2, 0)
                elif c == 1:
                    emit_ctx(1, 1)
                    emit_epi(1)
                    emit_ctx(2, 1)
                    emit_ctx(3, 0)
                    emit_ctx(3, 1)
                elif c == 2:
                    emit_ctx(2, 2)
                    emit_epi(2)
                    emit_ctx(3, 2)
                else:
                    emit_ctx(3, 3)
                    emit_epi(3)

    nc.compile()
    return nc


def _get_nc(reps=1):
    key = ("nc", reps)
    if key not in _CACHE:
        _CACHE[key] = _build_nc(reps)
    return _CACHE[key]


def kernel(q, k, v, key_padding_mask=None, Wq=None, bq=None, Wk=None, bk=None,
           Wv=None, bv=None):
    from concourse.bass_utils import run_bass_kernel_spmd

    nc = _get_nc()
    f = np.float32
    shared = {
        "wq": np.ascontiguousarray(Wq, dtype=f),
        "wk": np.ascontiguousarray(Wk, dtype=f),
        "wv": np.ascontiguousarray(Wv, dtype=f),
        "bq": np.ascontiguousarray(bq, dtype=f),
        "bk": np.ascontiguousarray(bk, dtype=f),
        "bv": np.ascontiguousarray(bv, dtype=f),
    }
    in_maps = []
    for n in range(NCORES):
        m = dict(shared)
        m["q"] = np.ascontiguousarray(q[n], dtype=f)
        m["k"] = np.ascontiguousarray(k[n], dtype=f)
        m["v"] = np.ascontiguousarray(v[n], dtype=f)
        in_maps.append(m)
    res = run_bass_kernel_spmd(nc, in_maps, core_ids=list(range(NCORES)))
    out = np.stack([res.results[i]["out"] for i in range(NCORES)], axis=0)
    return out.astype(np.float32)


# revision 72
# speedup vs baseline: 1.0291x; 1.0015x over previous
"""Causal attention layer (N=8, L=2048, H=1024, E=64) on 8 TRN2 NeuronCores.

Sharding: data-parallel over batch N - one batch element per core, Q/K/V
projection weights replicated. No collectives.

Per-core pipeline (93540 -> 55324 ns cost-model time vs the session-start
baseline):
  1. q/k cast-loaded f32->fp8e4m3 (SWDGE) in 512-row chunks; v f32->bf16.
     Load order k0,q0,k1,q1,... then v0..v3 so scores unlock earliest; the
     load stream runs gap-free on the DMA device.
  2. q/k chunks transposed as bf16-VIEWED fp8 PAIRS: 16 PE transposes per
     chunk (half of plain fp8) grouped 8-per-PSUM-bank, one evacuation copy
     per bank; layout [128(h-pair), 4(b), 512(l)] where partition p of
     block b holds h = 256b+2p+{0,1} interleaved.
  3. q/k projections as fp8 DoubleRow matmuls (contract 256 h per matmul,
     0.5 cyc/row): stationary wdr[b] = [128,2(t),64(e)] (de-interleaved W,
     built once via pair-transposes of W); bias added during PSUM evac;
     qpT/kpT stored fp8e4m3 [64, 2048].
  4. Scores ALSO DoubleRow at 0.5 cyc/row via stride-0 broadcast (both
     k-tiles alias the same data => logits doubled; the 1/2 is folded into
     the exp scale). exp on ScalarE; causal mask = upper-tri multiply on
     diagonal blocks; P^T tiles persist in SBUF (bf16). Score pairs are
     emitted lazily (pump) between other PE work so the in-order PE queue
     never head-blocks on the sc<->exp PSUM-recycling ladder.
  5. v0/v1: plain bf16 PE transposes (32/chunk, 8-per-bank groups) -> chv;
     v2/v3: one DmaTransposeAnt each, batched AFTER all loads (single
     DMACopy<->DmaTranspose mode transition; the DMA device idles then).
     Projection -> vpT bf16, PE-transposed (batched) to vaug [128,16,65]
     with a ones column (ctx matmul accumulates softmax row-sums free).
  6. ctx^T[65, i] += vaug[j].T @ P^T[j, i], stripes fully sequential on ONE
     PSUM accumulator (frees a bank so the score pipeline gets 3 PSUM bufs);
     batched epilogue PE-transposes back, divides by row-sum, DMAs out.
"""

import math

import numpy as np

N, L, H, E = 8, 2048, 1024, 64
NCORES = 8
CHUNK = 512
NCHUNK = L // CHUNK  # 4
TPC = CHUNK // 128  # 4 l-tiles per chunk
NBLK = L // 128  # 16 j-blocks
HB = H // 128  # 8

_CACHE = {}

import os
NXBAR_F = int(os.environ.get('KN_NXBAR', '2'))
CONSTS_FIRST = int(os.environ.get('KN_CONSTS_FIRST', '0'))
WCAST_ACT = int(os.environ.get('KN_WCAST_ACT', '0'))
PUMP_N = int(os.environ.get('KN_PUMP', '1'))
SEQCTX = int(os.environ.get('KN_SEQCTX', '1'))
WPREPV_DEFER = int(os.environ.get('KN_WPREPV', '0'))
VORDER = os.environ.get('KN_VORDER', '0123')
PJEVAC_DVE = int(os.environ.get('KN_PJEVAC', '0'))
DRAINC = int(os.environ.get('KN_DRAINC', '0'))
BUFS = [int(x) for x in os.environ.get('KN_BUFS', '6,4,3,3').split(',')]



def _build_nc(reps=1):
    from contextlib import ExitStack

    import concourse.mybir as mybir
    import concourse.tile as tile
    from concourse import bacc
    from concourse.masks import make_identity, make_upper_triangular
    from concourse.tile_rust import add_dep_helper

    f32 = mybir.dt.float32
    bf16 = mybir.dt.bfloat16
    fp8 = mybir.dt.float8e4
    AF = mybir.ActivationFunctionType
    DR = mybir.MatmulPerfMode.DoubleRow
    # DR stride-0 scores double the logit; fold the 1/2 into the exp scale
    scale = 1.0 / (2.0 * math.sqrt(float(L)))

    nc = bacc.Bacc("TRN2", target_bir_lowering=False, debug=False)

    q_ap = nc.dram_tensor("q", [L, H], f32, kind="ExternalInput").ap()
    k_ap = nc.dram_tensor("k", [L, H], f32, kind="ExternalInput").ap()
    v_ap = nc.dram_tensor("v", [L, H], f32, kind="ExternalInput").ap()
    wq_ap = nc.dram_tensor("wq", [E, H], f32, kind="ExternalInput").ap()
    wk_ap = nc.dram_tensor("wk", [E, H], f32, kind="ExternalInput").ap()
    wv_ap = nc.dram_tensor("wv", [E, H], f32, kind="ExternalInput").ap()
    bq_ap = nc.dram_tensor("bq", [E], f32, kind="ExternalInput").ap()
    bk_ap = nc.dram_tensor("bk", [E], f32, kind="ExternalInput").ap()
    bv_ap = nc.dram_tensor("bv", [E], f32, kind="ExternalInput").ap()
    out_ap = nc.dram_tensor("out", [L, E], f32, kind="ExternalOutput").ap()

    x_ap_of = {"q": q_ap, "k": k_ap, "v": v_ap}

    with tile.TileContext(nc) as tc, ExitStack() as ctx:
        const = ctx.enter_context(tc.tile_pool(name="const", bufs=1))
        pTsb = ctx.enter_context(tc.tile_pool(name="pTsb", bufs=1))
        nat8p = ctx.enter_context(tc.tile_pool(name="nat8", bufs=BUFS[0]))
        natvp = ctx.enter_context(tc.tile_pool(name="natv", bufs=BUFS[1]))
        trqp = ctx.enter_context(tc.tile_pool(name="trq", bufs=BUFS[2]))
        chvp = ctx.enter_context(tc.tile_pool(name="chv", bufs=BUFS[3]))
        ptp = ctx.enter_context(tc.tile_pool(name="pt", bufs=1))
        epip = ctx.enter_context(tc.tile_pool(name="epi", bufs=4))
        tpps = ctx.enter_context(tc.tile_pool(name="tpps", bufs=2, space="PSUM"))
        projps = ctx.enter_context(tc.tile_pool(name="projps", bufs=1, space="PSUM")) if SEQCTX != 2 else None
        scps = ctx.enter_context(tc.tile_pool(
            name="scps", bufs=(3 if SEQCTX else 2), space="PSUM"))
        ctxps = ctx.enter_context(tc.tile_pool(
            name="ctxps", bufs=(1 if SEQCTX in (1, 4) else 2), space="PSUM"))
        smallps = ctx.enter_context(tc.tile_pool(name="smallps", bufs=1, space="PSUM"))

        # ---- persistent tiles ----
        identf = const.tile([128, 128], f32, name="identf")
        identb = const.tile([128, 128], bf16, name="identb")
        tri_f32 = const.tile([128, 128], f32, name="tri_f32")
        tri = const.tile([128, 128], bf16, name="tri")
        b_sb = {}
        wf = {}
        for nm in ("q", "k", "v"):
            b_sb[nm] = const.tile([E, 1], f32, name=f"b_{nm}")
            wf[nm] = const.tile([E, H], f32, name=f"wf_{nm}")
        w8 = {nm: const.tile([E, H], fp8, name=f"w8_{nm}") for nm in ("q", "k")}
        wvb = const.tile([E, H], bf16, name="wvb")
        wdr = {nm: const.tile([128, 4, 2, E], fp8, name=f"wdr_{nm}")
               for nm in ("q", "k")}
        wTv = const.tile([128, HB, E], bf16, name="wTv")

        qpT8 = pTsb.tile([E, L], fp8, name="qpT8")
        kpT8 = pTsb.tile([E, L], fp8, name="kpT8")
        vpT = pTsb.tile([E, L], bf16, name="vpT")
        vaug = pTsb.tile([128, NBLK, E + 1], bf16, name="vaug")
        pT8_of = {"q": qpT8, "k": kpT8}

        def emit_w_loads():
            for nm, w_ap, bias_ap in (("q", wq_ap, bq_ap), ("k", wk_ap, bk_ap),
                                      ("v", wv_ap, bv_ap)):
                nc.scalar.dma_start(out=wf[nm][:], in_=w_ap)
                nc.scalar.dma_start(out=b_sb[nm][:], in_=bias_ap)

        def emit_consts():
            make_identity(nc, identf[:])
            nc.scalar.activation(identb[:], identf[:], AF.Identity)
            # tri[r, c] = 1 where c >= r (key row j <= query col i)
            make_upper_triangular(nc, tri_f32[:], val=1.0, diag=True)
            nc.scalar.activation(tri[:], tri_f32[:], AF.Identity)
            nc.vector.memset(vaug[:, :, E:E + 1], 1.0)

        def emit_w_prep_qk():
            # k's chain on DVE (feeds chunk k0 first), q's on Act; each
            # engine's queue stays short ahead of the first projections
            nc.vector.tensor_copy(w8["k"][:], wf["k"][:])
            nc.scalar.activation(w8["q"][:], wf["q"][:], AF.Identity)
            for nm, copy in (("k", nc.vector.tensor_copy),
                             ("q", lambda d, s: nc.scalar.activation(
                                 d, s, AF.Identity))):
                wu = w8[nm][:].bitcast(bf16)  # [64, 512]
                wps = tpps.tile([128, 2, TPC, 128], bf16, tag="tp",
                                name=f"wps_{nm}")
                for b in range(4):
                    nc.tensor.transpose(
                        wps[:, 0, b, 0:E], wu[:, b * 128:(b + 1) * 128],
                        identb[:E, :E])
                copy(
                    wdr[nm][:],
                    wps[:, 0, :, 0:E].bitcast(fp8).rearrange(
                        "p b (e t) -> p b t e", t=2))

        def emit_w_prep_v():
            nc.vector.tensor_copy(wvb[:], wf["v"][:])
            for g in range(2):
                wps = tpps.tile([128, 2, TPC, 128], bf16, tag="tp",
                                name=f"wvps{g}")
                for i in range(4):
                    hb = g * 4 + i
                    nc.tensor.transpose(
                        wps[:, i // 2, i % 2, 0:E],
                        wvb[:, hb * 128:(hb + 1) * 128], identb[:E, :E])
                nc.vector.tensor_copy(
                    wTv[:, g * 4:(g + 1) * 4].rearrange(
                        "p (a b) e -> p a b e", a=2),
                    wps[:, :, 0:2, 0:E])

        def emit_load(nm, c):
            l0 = c * CHUNK
            dtt = bf16 if nm == "v" else fp8
            pool = natvp if nm == "v" else nat8p
            nat = pool.tile([128, TPC, H], dtt, tag="nat", name=f"nat_{nm}{c}")
            src = x_ap_of[nm][l0:l0 + CHUNK, :].rearrange("(t p) h -> p t h", p=128)
            ld = nc.gpsimd.dma_start(out=nat[:].rearrange("p t h -> p (t h)"),
                                     in_=src)
            return nat, ld

        # ---- q/k chunk: pair transposes (2 groups of 8) + DR projection ----
        def emit_qk_tp_proj(nm, c, nat):
            l0 = c * CHUNK
            natu = nat[:].bitcast(bf16)  # [128, TPC, 512] u16 pairs
            trq = trqp.tile([128, 4, CHUNK], bf16, tag="trq", name=f"trq_{nm}{c}")
            for g in range(2):  # lt in {2g, 2g+1}
                tps = tpps.tile([128, 2, 4, 128], bf16, tag="tp",
                                name=f"tp_{nm}{c}{g}")
                for i in range(2):
                    lt = 2 * g + i
                    for b in range(4):
                        nc.tensor.transpose(
                            tps[:, i, b], natu[:, lt, b * 128:(b + 1) * 128],
                            identb[:])
                # [128, (lt2, b, l)] -> trq[:, b, (2g+i)*128 + l]
                nc.vector.tensor_copy(
                    trq[:, :, g * 256:(g + 1) * 256].rearrange(
                        "p b (i l) -> p i b l", i=2),
                    tps[:])
                pump(PUMP_N)
            if SEQCTX == 2:
                pj = scps.tile([128, 512], f32, tag="sc",
                               name=f"pj_{nm}{c}")[:E, :]
            else:
                pj = projps.tile([E + 1, CHUNK], f32, tag="pj",
                                 name=f"pj_{nm}{c}")[0:E, :]
            for b in range(4):
                nc.tensor.matmul(
                    pj[:],
                    lhsT=wdr[nm][:, b],
                    rhs=trq[:, b].bitcast(fp8).rearrange("p (l t) -> p t l", t=2),
                    start=(b == 0),
                    stop=(b == 3),
                    perf_mode=DR,
                )
            if PJEVAC_DVE:
                nc.vector.tensor_scalar_add(pT8_of[nm][:, l0:l0 + CHUNK],
                                            pj[:], b_sb[nm][:])
            else:
                nc.scalar.activation(pT8_of[nm][:, l0:l0 + CHUNK], pj[:],
                                     AF.Identity, bias=b_sb[nm][:])
            pump(1)

        # ---- scores (DoubleRow stride-0), emitted lazily via pump() so the
        # in-order PE queue never head-blocks on the exp ladder ----
        pt_info = {}
        pending = []  # (s, pi, take) score pairs not yet emitted

        def dr2(ap):
            return ap.rearrange("p (o l) -> p o l", o=1).broadcast_to(
                (ap.shape[0], 2, ap.shape[-1]))

        def queue_scores(s):
            i0, i1 = s * CHUNK, (s + 1) * CHUNK
            js = list(range(4 * s + 4))
            pi = 0
            pt_info[s] = []
            while js:
                take = js[:1]
                w0 = i1 - max(i0, js[0] * 128)
                if len(js) > 1 and w0 + (i1 - max(i0, js[1] * 128)) <= 512:
                    take = js[:2]
                pending.append((s, pi, take))
                pi += 1
                js = js[len(take):]

        def pump(n):
            for _ in range(min(n, len(pending))):
                s, pi, take = pending.pop(0)
                i0, i1 = s * CHUNK, (s + 1) * CHUNK
                sc = scps.tile([128, 512], f32, tag="sc", name=f"sc_{s}_{pi}")
                pt = ptp.tile([128, 512], bf16, tag=f"pt_{s}_{pi}",
                              name=f"pt_{s}_{pi}")
                infos = []
                off = 0
                for j in take:
                    g0 = max(i0, j * 128)
                    w = i1 - g0
                    nc.tensor.matmul(
                        sc[:, off:off + w],
                        lhsT=dr2(kpT8[:, j * 128:(j + 1) * 128]),
                        rhs=dr2(qpT8[:, g0:g0 + w]),
                        start=True,
                        stop=True,
                        perf_mode=DR,
                    )
                    infos.append((j, g0, w, off))
                    off += w
                nc.scalar.activation(pt[:, 0:off], sc[:, 0:off], AF.Exp,
                                     scale=scale)
                for j, g0, w, off_ in infos:
                    if g0 == j * 128:  # diagonal block: causal mask
                        nc.vector.tensor_mul(
                            pt[:, off_:off_ + 128], pt[:, off_:off_ + 128],
                            tri[:])
                pt_info[s].append((pt, infos))

        def drain_stripe(s):
            while pending and pending[0][0] <= s:
                pump(1)

        # ---- v chunk ----
        # c < NXBAR: plain PE transposes (Act/DVE have slack early).
        # c >= NXBAR: one DmaTranspose per chunk, batched AFTER all loads
        # (single DMACopy<->DmaTranspose mode transition; DMA idles then).
        NXBAR = NXBAR_F
        def emit_v_chunk(c, nat, last_ld, prev_xb):
            l0 = c * CHUNK
            xb = None
            if c >= NXBAR:
                cht = chvp.tile([128, HB * TPC, 128], bf16, tag="chx",
                                name=f"chx{c}")
                xb = nc.sync.dma_start(
                    out=cht[:],
                    in_=nat[:].rearrange("p t h -> p (t h)"),
                    transpose=True,
                )
                add_dep_helper(xb.ins, last_ld.ins, sync=True,
                               reason="batch v xbars after all loads")
                # block b of cht = (lt, hb): rhs for hb = [128, lt, 128]
                chb = cht[:].rearrange("p (lt hb) l -> p lt hb l", lt=TPC, hb=HB)
                rhs_of = lambda hb: chb[:, :, hb, :]
            else:
                chv = chvp.tile([128, HB, CHUNK], bf16, tag="chv",
                                name=f"chv{c}")
                for g in range(4):  # hb in {2g, 2g+1}
                    vt = tpps.tile([128, 2, TPC, 128], bf16, tag="tp",
                                   name=f"vt{c}{g}")
                    for i in range(2):
                        hb = 2 * g + i
                        for lt in range(TPC):
                            nc.tensor.transpose(
                                vt[:, i, lt],
                                nat[:, lt, hb * 128:(hb + 1) * 128],
                                identb[:])
                    nc.vector.tensor_copy(chv[:, 2 * g:2 * g + 2], vt[:])
                    pump(1)
                rhs_of = lambda hb: chv[:, hb]
            if SEQCTX == 2:
                pj = scps.tile([128, 512], f32, tag="sc",
                               name=f"pjv{c}")[:E, :]
            else:
                pj = projps.tile([E + 1, CHUNK], f32, tag="pj",
                                 name=f"pjv{c}")[0:E, :]
            for hb in range(HB):
                nc.tensor.matmul(
                    pj[:],
                    lhsT=wTv[:, hb],
                    rhs=rhs_of(hb),
                    start=(hb == 0),
                    stop=(hb == HB - 1),
                )
            nc.vector.tensor_scalar_add(vpT[:, l0:l0 + CHUNK], pj[:], b_sb["v"][:])
            # batched vaug: 4 transposes into one PSUM tile, one copy out
            sm = smallps.tile([128, 4, E + 4], f32, tag="sm", name=f"vaugtp{c}")
            vps = sm[:].bitcast(bf16)  # [128, 4, 2*(E+4)]
            for t in range(TPC):
                jb = c * TPC + t
                nc.tensor.transpose(
                    vps[:, t, :E], vpT[:, jb * 128:(jb + 1) * 128],
                    identb[:E, :E])
            nc.vector.tensor_copy(
                vaug[:, c * TPC:(c + 1) * TPC, 0:E], vps[:, :, 0:E])
            return xb

        # ---- ctx accumulation ----
        ctx_of = {}

        def emit_ctx(s, c):
            drain_stripe(s)
            assert not any(p[0] <= s for p in pending)
            i0 = s * CHUNK
            jmax = 4 * s + 3
            if s not in ctx_of:
                if SEQCTX == 4 and s == NCHUNK - 1:
                    ctx_of[s] = projps.tile([E + 1, CHUNK], f32, tag="pj",
                                            name=f"ctx{s}")
                else:
                    ctx_of[s] = ctxps.tile([E + 1, CHUNK], f32, tag="ctx",
                                           name=f"ctx{s}")
            ctx_ps = ctx_of[s]
            jlo, jhi = 4 * c, min(4 * c + 3, jmax)
            for pt, infos in pt_info[s]:
                for j, g0, w, off in infos:
                    if not (jlo <= j <= jhi):
                        continue
                    nc.tensor.matmul(
                        ctx_ps[:, g0 - i0:g0 - i0 + w],
                        lhsT=vaug[:, j],
                        rhs=pt[:, off:off + w],
                        start=(j == 0),
                        stop=(j == jmax),
                    )

        def emit_epi(s):
            i0, i1 = s * CHUNK, (s + 1) * CHUNK
            late = s >= 2
            ctx_ps = ctx_of[s]
            ctxsb = epip.tile([E + 1, CHUNK], bf16, tag="ctxsb", name=f"ctxsb{s}")
            nc.vector.tensor_copy(ctxsb[:], ctx_ps[:])
            outsb = epip.tile([128, TPC, E], f32, tag="outsb", name=f"outsb{s}")
            # batched epilogue in bf16 (transposes at 1 cyc/row vs f32's 2):
            # 4 transposes into one PSUM tile, one recip, one broadcast mul
            smf = smallps.tile([128, 4, E + 4], f32, tag="sm", name=f"etp{s}")
            cps = smf[:].bitcast(bf16)  # [128, 4, 2*(E+4)]
            for t in range(TPC):
                nc.tensor.transpose(
                    cps[:, t, 0:E + 1],
                    ctxsb[:, t * 128:(t + 1) * 128],
                    identb[:E + 1, :E + 1],
                )
            rec = epip.tile([128, TPC], f32, tag="rec", name=f"rec{s}")
            nc.vector.reciprocal(rec[:], cps[:, :, E:E + 1])
            # one broadcast multiply instead of four: rec stride-0 along e
            nc.vector.tensor_mul(
                outsb[:],
                cps[:, :, 0:E],
                rec[:].rearrange("p (t o) -> p t o", o=1).broadcast_to(
                    (128, TPC, E)))
            dst = out_ap[i0:i1, :].rearrange("(t p) e -> p t e", p=128)
            nc.scalar.dma_start(out=dst, in_=outsb[:])

        # ================= emission schedule =================
        for _ in range(reps):
            pt_info.clear()
            ctx_of.clear()
            del pending[:]
            if CONSTS_FIRST:
                emit_consts()  # Pool consts first: DMA init hides them
            emit_w_loads()
            nats = {}
            nats[("k", 0)] = emit_load("k", 0)[0]
            nats[("q", 0)] = emit_load("q", 0)[0]
            if not CONSTS_FIRST:
                emit_consts()
            emit_w_prep_qk()
            if not WPREPV_DEFER:
                emit_w_prep_v()
            for c in range(NCHUNK):
                if c > 0:
                    nats[("k", c)] = emit_load("k", c)[0]
                    nats[("q", c)] = emit_load("q", c)[0]
                emit_qk_tp_proj("k", c, nats[("k", c)])
                emit_qk_tp_proj("q", c, nats[("q", c)])
                queue_scores(c)
                if DRAINC == -2:
                    drain_stripe(c)  # emit each stripe's scores immediately
                if c == 0 and WPREPV_DEFER:
                    emit_w_prep_v()
            if DRAINC < 0:
                drain_stripe(NCHUNK - 1)  # drain all scores pre-v-phase
            vorder = [int(x) for x in VORDER]
            vload_map = {}
            for c in vorder:
                vload_map[c] = emit_load("v", c)
            vloads = [vload_map[c] for c in range(NCHUNK)]
            last_ld = vload_map[vorder[-1]][1]
            # v blocks with interleaved ctx; stripe pairing keeps ctxps at 2
            # bufs without in-order-queue deadlock (epi(s) is emitted before
            # any stripe that reuses its PSUM accumulator).
            prev_xb = None
            for c in range(NCHUNK):
                prev_xb = emit_v_chunk(c, vloads[c][0], last_ld, prev_xb)
                if SEQCTX in (1, 4):
                    # stripes sequential on one ctx accumulator; in mode 4
                    # the LAST stripe instead accumulates in the projection
                    # bank (free once v3's projection evacuates), in
                    # parallel with stripe 2's chain
                    if SEQCTX == 4 and c == NCHUNK - 1:
                        pass  # stripe 3 handled below on projps
                    else:
                        for cc in range(c + 1):
                            emit_ctx(c, cc)
                        emit_epi(c)
                    if SEQCTX == 4 and c == NCHUNK - 1:
                        for cc in range(NCHUNK):
                            emit_ctx(NCHUNK - 1, cc)
                        emit_epi(NCHUNK - 1)
                    if c == max(DRAINC, 0):
                        # all remaining score pairs must be in the PE queue
                        # BEFORE the v2/v3 xbar-gated work, or their exps
                        # gate the final stripe's context tail
                        drain_stripe(NCHUNK - 1)
                elif c == 0:
                    emit_ctx(0, 0)
                    emit_epi(0)
                    emit_ctx(1, 0)
                    emit_ctx(2, 0)
                elif c == 1:
                    emit_ctx(1, 1)
                    emit_epi(1)
                    emit_ctx(2, 1)
                    emit_ctx(3, 0)
                    emit_ctx(3, 1)
                elif c == 2:
                    emit_ctx(2, 2)
                    emit_epi(2)
                    emit_ctx(3, 2)
                else:
                    emit_ctx(3, 3)
                    emit_epi(3)

    nc.compile()
    return nc


def _get_nc(reps=1):
    key = ("nc", reps)
    if key not in _CACHE:
        _CACHE[key] = _build_nc(reps)
    return _CACHE[key]


def kernel(q, k, v, key_padding_mask=None, Wq=None, bq=None, Wk=None, bk=None,
           Wv=None, bv=None):
    from concourse.bass_utils import run_bass_kernel_spmd

    nc = _get_nc()
    f = np.float32
    shared = {
        "wq": np.ascontiguousarray(Wq, dtype=f),
        "wk": np.ascontiguousarray(Wk, dtype=f),
        "wv": np.ascontiguousarray(Wv, dtype=f),
        "bq": np.ascontiguousarray(bq, dtype=f),
        "bk": np.ascontiguousarray(bk, dtype=f),
        "bv": np.ascontiguousarray(bv, dtype=f),
    }
    in_maps = []
    for n in range(NCORES):
        m = dict(shared)
        m["q"] = np.ascontiguousarray(q[n], dtype=f)
        m["k"] = np.ascontiguousarray(k[n], dtype=f)
        m["v"] = np.ascontiguousarray(v[n], dtype=f)
        in_maps.append(m)
    res = run_bass_kernel_spmd(nc, in_maps, core_ids=list(range(NCORES)))
    out = np.stack([res.results[i]["out"] for i in range(NCORES)], axis=0)
    return out.astype(np.float32)
